# revision 3
# baseline (speedup 1.0000x reference)
"""KV-cache sliding-window update for Trainium2 (Bass), 8-core SPMD.

Reference semantics (per batch b, head h):
    C = concat([cache, new], time)                  # [T + T_NEW]
    out = concat([C[:SINK], C[-WINDOW:]], time)     # [SINK + WINDOW]

With T=4096, T_NEW=16, WINDOW=4096, SINK=4 this is pure data movement:
    out[0:4]      = cache[0:4]        (sink tokens)
    out[4:4084]   = cache[16:4096]    (kept window, 4080 rows)
    out[4084:4100]= new[0:16]         (new tokens)

Each (b, h) row is independent, so we shard the flattened (B*H) = 128 rows
across 8 NeuronCores (16 rows each; equivalent to batch x head-half tensor
parallel). Per core the NEFF is just 6 DRAM->DRAM DMA copies (3 per K/V
tensor) issued on the HWDGE queue — no SBUF staging, no compute.
"""

import numpy as np

import concourse.bass as bass
import concourse.mybir as mybir
from concourse.bass_utils import run_bass_kernel_spmd

B, H, T, T_NEW, D = 4, 32, 4096, 16, 128
WINDOW, SINK = 4096, 4
T_OUT = SINK + WINDOW            # 4100
MID_START = T + T_NEW - WINDOW   # 16: first kept row of the old cache
MID = T - MID_START              # 4080 kept rows
N_CORES = 8
R = B * H                        # 128 independent (b, h) rows
R_LOC = R // N_CORES             # 16 rows per core

TRACE = False          # test.py flips this to capture an NTFF profile
LAST_RESULTS = None    # BassKernelResults of the most recent run (for test.py)

_NC = None


def _build_nc():
    nc = bass.Bass()
    f32 = mybir.dt.float32
    k = nc.dram_tensor("K", [R_LOC, T, D], f32, kind="ExternalInput")
    v = nc.dram_tensor("V", [R_LOC, T, D], f32, kind="ExternalInput")
    kn = nc.dram_tensor("K_new", [R_LOC, T_NEW, D], f32, kind="ExternalInput")
    vn = nc.dram_tensor("V_new", [R_LOC, T_NEW, D], f32, kind="ExternalInput")
    ko = nc.dram_tensor("K_out", [R_LOC, T_OUT, D], f32, kind="ExternalOutput")
    vo = nc.dram_tensor("V_out", [R_LOC, T_OUT, D], f32, kind="ExternalOutput")

    # Three DMA queues (Sync HWDGE, Scalar HWDGE, GpSimd SWDGE): each SDMA
    # engine round-robins between the queues it has work on, overlapping one
    # descriptor's HBM read/write turnaround with another's. The kept-window
    # bulk (16 chunk-rows per tensor) is split by row across the queues.
    S_SYNC = slice(0, 6)       # rows 0:6  -> Sync queue
    S_SCAL = slice(6, 11)      # rows 6:11 -> Scalar queue
    S_GPS = slice(11, R_LOC)   # rows 11:16 -> GpSimd queue

    with nc.Block() as block, nc.semaphore("dma_sem") as sem, nc.semaphore(
        "dma_sem2"
    ) as sem2, nc.semaphore("dma_sem3") as sem3:

        @block.sync
        def _(sync):
            n = 0
            for src, new, dst in ((k, kn, ko), (v, vn, vo)):
                sync.dma_start(
                    dst[S_SYNC, SINK : SINK + MID, :], src[S_SYNC, MID_START:T, :]
                ).then_inc(sem, 16)
                # sink tokens: old rows [0, 4) -> out rows [0, 4)
                sync.dma_start(dst[:, 0:SINK, :], src[:, 0:SINK, :]).then_inc(sem, 16)
                n += 2
            sync.wait_ge(sem, 16 * n)

        @block.scalar
        def _(scalar):
            n = 0
            for src, new, dst in ((k, kn, ko), (v, vn, vo)):
                scalar.dma_start(
                    dst[S_SCAL, SINK : SINK + MID, :], src[S_SCAL, MID_START:T, :]
                ).then_inc(sem2, 16)
                # new tokens -> out rows [4084, 4100)
                scalar.dma_start(
                    dst[:, SINK + MID : T_OUT, :], new[:, :, :]
                ).then_inc(sem2, 16)
                n += 2
            scalar.wait_ge(sem2, 16 * n)

        @block.gpsimd
        def _(gpsimd):
            n = 0
            for src, new, dst in ((k, kn, ko), (v, vn, vo)):
                gpsimd.dma_start(
                    dst[S_GPS, SINK : SINK + MID, :], src[S_GPS, MID_START:T, :]
                ).then_inc(sem3, 16)
                n += 1
            gpsimd.wait_ge(sem3, 16 * n)

    return nc


def kernel(K, V, K_new, V_new):
    global _NC, LAST_RESULTS
    if _NC is None:
        _NC = _build_nc()

    ins = {
        "K": np.asarray(K, dtype=np.float32).reshape(R, T, D),
        "V": np.asarray(V, dtype=np.float32).reshape(R, T, D),
        "K_new": np.asarray(K_new, dtype=np.float32).reshape(R, T_NEW, D),
        "V_new": np.asarray(V_new, dtype=np.float32).reshape(R, T_NEW, D),
    }
    in_maps = [
        {name: arr[c * R_LOC : (c + 1) * R_LOC] for name, arr in ins.items()}
        for c in range(N_CORES)
    ]
    LAST_RESULTS = run_bass_kernel_spmd(
        _NC, in_maps, core_ids=list(range(N_CORES)), trace=TRACE
    )
    res = LAST_RESULTS.results
    K_out = np.concatenate([r["K_out"] for r in res], axis=0).reshape(B, H, T_OUT, D)
    V_out = np.concatenate([r["V_out"] for r in res], axis=0).reshape(B, H, T_OUT, D)
    return K_out, V_out


# revision 4
# speedup vs baseline: 1.0514x; 1.0514x over previous
"""KV-cache sliding-window update for Trainium2 (Bass), 8-core SPMD.

Reference semantics (per batch b, head h):
    C = concat([cache, new], time)                  # [T + T_NEW]
    out = concat([C[:SINK], C[-WINDOW:]], time)     # [SINK + WINDOW]

With T=4096, T_NEW=16, WINDOW=4096, SINK=4 this is pure data movement:
    out[0:4]      = cache[0:4]        (sink tokens)
    out[4:4084]   = cache[16:4096]    (kept window, 4080 rows)
    out[4084:4100]= new[0:16]         (new tokens)

Each (b, h) row is independent, so we shard the flattened (B*H) = 128 rows
across 8 NeuronCores (16 rows each; equivalent to batch x head-half tensor
parallel). Per core the NEFF is just 6 DRAM->DRAM DMA copies (3 per K/V
tensor) issued on the HWDGE queue — no SBUF staging, no compute.
"""

import numpy as np

import concourse.bass as bass
import concourse.mybir as mybir
from concourse.bass_utils import run_bass_kernel_spmd

B, H, T, T_NEW, D = 4, 32, 4096, 16, 128
WINDOW, SINK = 4096, 4
T_OUT = SINK + WINDOW            # 4100
MID_START = T + T_NEW - WINDOW   # 16: first kept row of the old cache
MID = T - MID_START              # 4080 kept rows
N_CORES = 8
R = B * H                        # 128 independent (b, h) rows
R_LOC = R // N_CORES             # 16 rows per core

TRACE = False          # test.py flips this to capture an NTFF profile
LAST_RESULTS = None    # BassKernelResults of the most recent run (for test.py)

_NC = None


def _build_nc():
    nc = bass.Bass()
    f32 = mybir.dt.float32
    k = nc.dram_tensor("K", [R_LOC, T, D], f32, kind="ExternalInput")
    v = nc.dram_tensor("V", [R_LOC, T, D], f32, kind="ExternalInput")
    kn = nc.dram_tensor("K_new", [R_LOC, T_NEW, D], f32, kind="ExternalInput")
    vn = nc.dram_tensor("V_new", [R_LOC, T_NEW, D], f32, kind="ExternalInput")
    ko = nc.dram_tensor("K_out", [R_LOC, T_OUT, D], f32, kind="ExternalOutput")
    vo = nc.dram_tensor("V_out", [R_LOC, T_OUT, D], f32, kind="ExternalOutput")

    # Two DMA queues (Sync + Scalar HWDGE rings): each SDMA engine interleaves
    # descriptors from both queues, overlapping one queue's HBM read/write
    # turnaround with the other's — measured 1.33x over a single queue.
    # (A third gpsimd/SWDGE queue measured slower: Q7 descriptor generation
    # shares SBUF ports with the SDMA engine hosting the rings.)
    # K's bulk goes on Sync, V's bulk on Scalar; the small sink/new copies
    # are cross-assigned so both queues carry exactly half the bytes.
    with nc.Block() as block, nc.semaphore("dma_sem") as sem, nc.semaphore(
        "dma_sem2"
    ) as sem2:

        @block.sync
        def _(sync):
            # kept window of K: old rows [16, 4096) -> out rows [4, 4084)
            sync.dma_start(
                ko[:, SINK : SINK + MID, :], k[:, MID_START:T, :]
            ).then_inc(sem, 16)
            # V sink tokens + V new tokens
            sync.dma_start(vo[:, 0:SINK, :], v[:, 0:SINK, :]).then_inc(sem, 16)
            sync.dma_start(vo[:, SINK + MID : T_OUT, :], vn[:, :, :]).then_inc(
                sem, 16
            )
            sync.wait_ge(sem, 48)

        @block.scalar
        def _(scalar):
            # kept window of V
            scalar.dma_start(
                vo[:, SINK : SINK + MID, :], v[:, MID_START:T, :]
            ).then_inc(sem2, 16)
            # K sink tokens + K new tokens
            scalar.dma_start(ko[:, 0:SINK, :], k[:, 0:SINK, :]).then_inc(sem2, 16)
            scalar.dma_start(ko[:, SINK + MID : T_OUT, :], kn[:, :, :]).then_inc(
                sem2, 16
            )
            scalar.wait_ge(sem2, 48)

    return nc


def kernel(K, V, K_new, V_new):
    global _NC, LAST_RESULTS
    if _NC is None:
        _NC = _build_nc()

    ins = {
        "K": np.asarray(K, dtype=np.float32).reshape(R, T, D),
        "V": np.asarray(V, dtype=np.float32).reshape(R, T, D),
        "K_new": np.asarray(K_new, dtype=np.float32).reshape(R, T_NEW, D),
        "V_new": np.asarray(V_new, dtype=np.float32).reshape(R, T_NEW, D),
    }
    in_maps = [
        {name: arr[c * R_LOC : (c + 1) * R_LOC] for name, arr in ins.items()}
        for c in range(N_CORES)
    ]
    LAST_RESULTS = run_bass_kernel_spmd(
        _NC, in_maps, core_ids=list(range(N_CORES)), trace=TRACE
    )
    res = LAST_RESULTS.results
    K_out = np.concatenate([r["K_out"] for r in res], axis=0).reshape(B, H, T_OUT, D)
    V_out = np.concatenate([r["V_out"] for r in res], axis=0).reshape(B, H, T_OUT, D)
    return K_out, V_out


# revision 5
# speedup vs baseline: 1.2003x; 1.1416x over previous
"""KV-cache sliding-window update for Trainium2 (Bass), 8-core SPMD.

Reference semantics (per batch b, head h):
    C = concat([cache, new], time)                  # [T + T_NEW]
    out = concat([C[:SINK], C[-WINDOW:]], time)     # [SINK + WINDOW]

With T=4096, T_NEW=16, WINDOW=4096, SINK=4 this is pure data movement:
    out[0:4]      = cache[0:4]        (sink tokens)
    out[4:4084]   = cache[16:4096]    (kept window, 4080 rows)
    out[4084:4100]= new[0:16]         (new tokens)

Each (b, h) row is independent, so we shard the flattened (B*H) = 128 rows
across 8 NeuronCores (16 rows each; equivalent to batch x head-half tensor
parallel). Per core the NEFF is just 6 DRAM->DRAM DMA copies (3 per K/V
tensor) issued on the HWDGE queue — no SBUF staging, no compute.
"""

import numpy as np

import concourse.bass as bass
import concourse.mybir as mybir
from concourse.bass_utils import run_bass_kernel_spmd

B, H, T, T_NEW, D = 4, 32, 4096, 16, 128
WINDOW, SINK = 4096, 4
T_OUT = SINK + WINDOW            # 4100
MID_START = T + T_NEW - WINDOW   # 16: first kept row of the old cache
MID = T - MID_START              # 4080 kept rows
N_CORES = 8
R = B * H                        # 128 independent (b, h) rows
R_LOC = R // N_CORES             # 16 rows per core

TRACE = False          # test.py flips this to capture an NTFF profile
LAST_RESULTS = None    # BassKernelResults of the most recent run (for test.py)

_NC = None


def _build_nc():
    # enable_partition_id=False drops the per-engine TENSOR_LOAD preamble
    # (~5 us) — this kernel is SPMD by data only and never reads the core id.
    nc = bass.Bass(enable_partition_id=False)
    f32 = mybir.dt.float32
    k = nc.dram_tensor("K", [R_LOC, T, D], f32, kind="ExternalInput")
    v = nc.dram_tensor("V", [R_LOC, T, D], f32, kind="ExternalInput")
    kn = nc.dram_tensor("K_new", [R_LOC, T_NEW, D], f32, kind="ExternalInput")
    vn = nc.dram_tensor("V_new", [R_LOC, T_NEW, D], f32, kind="ExternalInput")
    ko = nc.dram_tensor("K_out", [R_LOC, T_OUT, D], f32, kind="ExternalOutput")
    vo = nc.dram_tensor("V_out", [R_LOC, T_OUT, D], f32, kind="ExternalOutput")

    # Two DMA queues (Sync + Scalar HWDGE rings): each SDMA engine interleaves
    # descriptors from both queues, overlapping one queue's HBM read/write
    # turnaround with the other's — measured 1.33x over a single queue.
    # (A third gpsimd/SWDGE queue measured slower: Q7 descriptor generation
    # shares SBUF ports with the SDMA engine hosting the rings.)
    # K's bulk goes on Sync, V's bulk on Scalar; the small sink/new copies
    # are cross-assigned so both queues carry exactly half the bytes.
    with nc.Block() as block, nc.semaphore("dma_sem") as sem, nc.semaphore(
        "dma_sem2"
    ) as sem2:

        @block.sync
        def _(sync):
            # kept window of K: old rows [16, 4096) -> out rows [4, 4084)
            sync.dma_start(
                ko[:, SINK : SINK + MID, :], k[:, MID_START:T, :]
            ).then_inc(sem, 16)
            # V sink tokens + V new tokens
            sync.dma_start(vo[:, 0:SINK, :], v[:, 0:SINK, :]).then_inc(sem, 16)
            sync.dma_start(vo[:, SINK + MID : T_OUT, :], vn[:, :, :]).then_inc(
                sem, 16
            )
            sync.wait_ge(sem, 48)

        @block.scalar
        def _(scalar):
            # kept window of V
            scalar.dma_start(
                vo[:, SINK : SINK + MID, :], v[:, MID_START:T, :]
            ).then_inc(sem2, 16)
            # K sink tokens + K new tokens
            scalar.dma_start(ko[:, 0:SINK, :], k[:, 0:SINK, :]).then_inc(sem2, 16)
            scalar.dma_start(ko[:, SINK + MID : T_OUT, :], kn[:, :, :]).then_inc(
                sem2, 16
            )
            scalar.wait_ge(sem2, 48)

    return nc


def kernel(K, V, K_new, V_new):
    global _NC, LAST_RESULTS
    if _NC is None:
        _NC = _build_nc()

    ins = {
        "K": np.asarray(K, dtype=np.float32).reshape(R, T, D),
        "V": np.asarray(V, dtype=np.float32).reshape(R, T, D),
        "K_new": np.asarray(K_new, dtype=np.float32).reshape(R, T_NEW, D),
        "V_new": np.asarray(V_new, dtype=np.float32).reshape(R, T_NEW, D),
    }
    in_maps = [
        {name: arr[c * R_LOC : (c + 1) * R_LOC] for name, arr in ins.items()}
        for c in range(N_CORES)
    ]
    LAST_RESULTS = run_bass_kernel_spmd(
        _NC, in_maps, core_ids=list(range(N_CORES)), trace=TRACE
    )
    res = LAST_RESULTS.results
    K_out = np.concatenate([r["K_out"] for r in res], axis=0).reshape(B, H, T_OUT, D)
    V_out = np.concatenate([r["V_out"] for r in res], axis=0).reshape(B, H, T_OUT, D)
    return K_out, V_out
